# revision 5
# baseline (speedup 1.0000x reference)
"""Trainium2 Bass kernel v5 for BERT4ETH adjacency build (FOLD design).

v3/v4 lineage (see kernel_v3.py docstring for the fold layout), plus:
  - bf16 features/values on the wire (halves the dominant DMA term);
    d is still accumulated in f32, so only input quantization error
    (~1e-3 relative) is introduced.
  - three-engine pipeline per chunk: DVE does the 5 weighted-feature
    MAC passes (scalar-AP ops are DVE-only in walrus), GPSIMD (Pool)
    does the value multiply and all the fold adds (plain tensor_tensor
    ops), ACT compacts the results into the accumulator.
  - triple-buffered input slots, NCHUNK=8 big chunks (per-chunk fixed
    costs — sem propagation, DMA issue, cross-engine stalls — amortize
    over 2x more work than v5).
  - features and value merged into ONE interleaved stream (6 bf16 per
    edge slot: f0..f4, v), so each chunk needs a single input DMA.
"""

import numpy as np
import ml_dtypes

import concourse.bass as bass
import concourse.mybir as mybir
from concourse.bass_utils import run_bass_kernel_spmd

F32 = mybir.dt.float32
BF16 = mybir.dt.bfloat16

N_CORES = 8
NUM_NODES = 3_000_000
N_FEAT = 5
NLANE = N_FEAT + 1            # interleaved lanes per edge slot: f0..f4, v
NCHUNK = 8
NBUF = 2
MAXD = 6
# class order in the column layout: Pool-owned classes first (contiguous
# prefix), DVE-owned big classes {2,3} last
ORDER = [0, 1, 4, 5, 2, 3]
POOL_N = 4                    # first POOL_N classes of ORDER run on Pool
S_D = [12, 24, 100, 206, 44, 2]  # per-(chunk, partition) node capacity
W_D = [1 << d for d in range(MAXD)]

_widths = [W_D[D] * S_D[D] for D in ORDER]
_offs = np.concatenate([[0], np.cumsum(_widths)])
OFF_D = {D: int(_offs[i]) for i, D in enumerate(ORDER)}
END_D = {D: int(_offs[i + 1]) for i, D in enumerate(ORDER)}
K = int(_offs[-1])            # leaf columns per chunk
POOL_END = int(_offs[POOL_N])  # Pool owns cols [0, POOL_END)
SUM_S = int(sum(S_D))
ACC_BASE = {D: int(np.cumsum([0] + [S_D[d] for d in ORDER])[i])
            for i, D in enumerate(ORDER)}
COLS = NCHUNK * K
ACC = NCHUNK * SUM_S


def _emit_macs(eng, V, pcnt_ref, wait_ge, A, Bs, fg, w_sb):
    """A = sum_f w_f f_f in bf16.  All 5 tensor_scalar products go to
    independent buffers and issue back-to-back (engine stays fed); the 4
    adds use TARGETED wait counts (product f + previous add only), so the
    serial critical path is just the add chain."""
    base = pcnt_ref[0]
    V(eng.tensor_scalar(
        A, fg[:, 0:K], w_sb[:, 0:1], None, mybir.AluOpType.mult,
    ))
    for f in range(1, N_FEAT):
        V(eng.tensor_scalar(
            Bs[f - 1], fg[:, f * K : (f + 1) * K], w_sb[:, f : f + 1], None,
            mybir.AluOpType.mult,
        ))
    for f in range(1, N_FEAT):
        # add_f needs ts_b_f (count base+1+f) and add_{f-1} (count
        # base+N_FEAT+f-1, emitted after all N_FEAT products)
        if f == 1:
            wait_ge(base + 2)
        else:
            wait_ge(base + N_FEAT + f - 1)
        V(eng.tensor_tensor(
            out=A, in0=A, in1=Bs[f - 1], op=mybir.AluOpType.add,
        ))


def _emit_folds(eng, V, Wt, X, classes, done_sem):
    """Strided fold adds for `classes`; last op incs done_sem instead."""
    folds = [
        (k, D)
        for k in range(1, MAXD)
        for D in classes
        if D >= k and S_D[D] > 0
    ]
    prev_k = 0
    for i, (k, D) in enumerate(folds):
        if k != prev_k:
            Wt()
            prev_k = k
        step, half = 1 << k, 1 << (k - 1)
        off, end = OFF_D[D], END_D[D]
        inst = eng.tensor_tensor(
            out=X[:, off:end:step],
            in0=X[:, off:end:step],
            in1=X[:, off + half : end : step],
            op=mybir.AluOpType.add,
        )
        if i == len(folds) - 1:
            inst.then_inc(done_sem, 1)
        else:
            V(inst)
    if not folds:  # degenerate: still signal
        Wt()
        eng.tensor_copy(X[:, 0:1], X[:, 0:1]).then_inc(done_sem, 1)


def build_nc(nchunk=NCHUNK, repeat=1):
    nc = bass.Bass()
    data = nc.dram_tensor("data", [128, COLS * NLANE], BF16, kind="ExternalInput")
    wvec = nc.dram_tensor("wvec", [128, 8], F32, kind="ExternalInput")
    out = nc.dram_tensor("out", [128, ACC], F32, kind="ExternalOutput")

    from contextlib import ExitStack
    ctx = ExitStack()
    with ctx:
        w_sb = ctx.enter_context(nc.sbuf_tensor("w_sb", [128, 8], F32))
        f_all = ctx.enter_context(
            nc.sbuf_tensor("f_sb", [128, NBUF * K * NLANE], BF16)
        )
        d_all = ctx.enter_context(nc.sbuf_tensor("d_sb", [128, NBUF * K], F32))
        a_all = ctx.enter_context(nc.sbuf_tensor("a_sb", [128, NBUF * K], BF16))
        b_all = ctx.enter_context(
            nc.sbuf_tensor("b_sb", [128, (N_FEAT - 1) * K], BF16)
        )
        accum = ctx.enter_context(nc.sbuf_tensor("accum", [128, ACC], F32))
        f_sb = [
            f_all[:, i * K * NLANE : (i + 1) * K * NLANE] for i in range(NBUF)
        ]
        d_sb = [d_all[:, i * K : (i + 1) * K] for i in range(NBUF)]
        a_sb = [a_all[:, i * K : (i + 1) * K] for i in range(NBUF)]
        b_sb = [b_all[:, i * K : (i + 1) * K] for i in range(N_FEAT - 1)]
        s_const = ctx.enter_context(nc.semaphore("s_const"))
        s_din = ctx.enter_context(nc.semaphore("s_din"))
        s_prep = ctx.enter_context(nc.semaphore("s_prep"))
        s_prepp = ctx.enter_context(nc.semaphore("s_prepp"))
        s_foldp = ctx.enter_context(nc.semaphore("s_foldp"))
        s_done = ctx.enter_context(nc.semaphore("s_done"))
        s_dout = ctx.enter_context(nc.semaphore("s_dout"))
        block = ctx.enter_context(nc.Block())

        @block.sync
        def _(sync):
            sync.dma_start(out=w_sb[:], in_=wvec[:]).then_inc(s_const, 16)
            for cc in range(nchunk * repeat):
                c = cc % nchunk
                s = cc % NBUF
                if cc >= NBUF:
                    sync.wait_ge(s_done, cc - NBUF + 1)
                sync.dma_start(
                    out=f_sb[s],
                    in_=data[:, c * K * NLANE : (c + 1) * K * NLANE],
                ).then_inc(s_din, 16)
            sync.wait_ge(s_done, nchunk * repeat)
            sync.dma_start(out=out[:], in_=accum[:]).then_inc(s_dout, 16)
            sync.wait_ge(s_dout, 16)

        @block.vector
        def _(vector):
            pcnt = 0

            def V(inst):
                nonlocal pcnt
                inst.then_inc(s_prep, 1)
                pcnt += 1

            def Wt():
                vector.wait_ge(s_prep, pcnt)

            vector.wait_ge(s_const, 16)
            for cc in range(nchunk * repeat):
                s = cc % NBUF
                vector.wait_ge(s_din, 16 * (cc + 1))
                _emit_macs(nc.vector, V, [pcnt],
                           lambda n: vector.wait_ge(s_prep, n),
                           a_sb[s], b_sb, f_sb[s], w_sb)
            assert pcnt == (2 * N_FEAT - 1) * nchunk * repeat

        @block.gpsimd
        def _(gpsimd):
            qcnt = 0

            def V(inst):
                nonlocal qcnt
                inst.then_inc(s_prepp, 1)
                qcnt += 1

            def Wt():
                gpsimd.wait_ge(s_prepp, qcnt)

            all_classes = [D for D in ORDER if S_D[D] > 0]
            for cc in range(nchunk * repeat):
                s = cc % NBUF
                # all 5 MAC passes of chunk cc retired on DVE
                gpsimd.wait_ge(s_prep, (2 * N_FEAT - 1) * (cc + 1))
                X = d_sb[s]
                V(nc.gpsimd.tensor_tensor(
                    out=X, in0=a_sb[s],
                    in1=f_sb[s][:, N_FEAT * K : NLANE * K],
                    op=mybir.AluOpType.mult
                ))
                _emit_folds(nc.gpsimd, V, Wt, X, all_classes, s_foldp)

        @block.scalar
        def _(scalar):
            last_D = ORDER[-1]
            for cc in range(nchunk * repeat):
                c = cc % nchunk
                s = cc % NBUF
                scalar.wait_ge(s_foldp, cc + 1)
                X = d_sb[s]
                aoff = c * SUM_S
                for D in ORDER:
                    if S_D[D] == 0:
                        continue
                    so = aoff + ACC_BASE[D]
                    inst = nc.scalar.copy(
                        accum[:, so : so + S_D[D]],
                        X[:, OFF_D[D] : END_D[D] : W_D[D]],
                    )
                    if D == last_D:
                        inst.then_inc(s_done, 1)

    return nc


# ---------------------------------------------------------------------------
# Host-side packing / unpacking
# ---------------------------------------------------------------------------

def _depth(c):
    d = np.zeros_like(c)
    for D in range(1, MAXD):
        d[c > (1 << (D - 1))] = D
    return d


def pack(rows):
    rows = np.asarray(rows, dtype=np.int64)
    counts = np.bincount(rows, minlength=NUM_NODES).astype(np.int64)
    if counts.max() > (1 << (MAXD - 1)):
        raise RuntimeError(f"node count {counts.max()} exceeds max class width")

    order = np.argsort(-counts, kind="stable")
    node_core = np.empty(NUM_NODES, dtype=np.int64)
    node_core[order] = np.arange(NUM_NODES) % N_CORES

    depth = _depth(counts)

    node_chunk = np.empty(NUM_NODES, dtype=np.int64)
    node_part = np.empty(NUM_NODES, dtype=np.int64)
    node_idx = np.empty(NUM_NODES, dtype=np.int64)
    for core in range(N_CORES):
        sel_core = node_core == core
        for D in range(MAXD):
            nodes = order[sel_core[order] & (depth[order] == D)]
            n = len(nodes)
            if n > S_D[D] * 128 * NCHUNK:
                raise RuntimeError(
                    f"core {core} class {D}: {n} nodes > capacity "
                    f"{S_D[D] * 128 * NCHUNK}"
                )
            r = np.arange(n)
            node_part[nodes] = r % 128
            node_chunk[nodes] = (r // 128) % NCHUNK
            node_idx[nodes] = r // (128 * NCHUNK)

    off_arr = np.array([OFF_D[D] for D in range(MAXD)])
    accb_arr = np.array([ACC_BASE[D] for D in range(MAXD)])
    node_leafcol = (
        node_chunk * K + off_arr[depth] + node_idx * np.array(W_D)[depth]
    )
    node_acccol = node_chunk * SUM_S + accb_arr[depth] + node_idx

    eorder = np.argsort(rows, kind="stable")
    rs = rows[eorder]
    starts = np.zeros(NUM_NODES, dtype=np.int64)
    starts[1:] = np.cumsum(counts)[:-1]
    j = np.arange(len(rows), dtype=np.int64) - starts[rs]
    ecol = node_leafcol[rs] + j
    epart = node_part[rs]
    gpos = node_core[rs] * (128 * COLS) + epart * COLS + ecol

    label = node_core * (128 * ACC) + node_part * ACC + node_acccol
    return eorder, gpos, label


def make_in_maps(features, values, a0_weight, rows):
    features = np.asarray(features, dtype=np.float32)
    values = np.asarray(values, dtype=np.float32)
    eorder, gpos, label = pack(rows)

    ecore = gpos // (128 * COLS)
    epart = (gpos // COLS) % 128
    ecol = gpos % COLS
    echunk = ecol // K
    ej = ecol % K
    data_all = np.zeros((N_CORES, 128, NCHUNK, NLANE, K), dtype=np.float32)
    fsrc = features[eorder]
    for f in range(N_FEAT):
        data_all[ecore, epart, echunk, f, ej] = fsrc[:, f]
    data_all[ecore, epart, echunk, N_FEAT, ej] = values[eorder]
    data_all = data_all.astype(ml_dtypes.bfloat16).reshape(
        N_CORES, 128, COLS * NLANE
    )

    w8 = np.zeros(8, dtype=np.float32)
    w8[:N_FEAT] = np.asarray(a0_weight, dtype=np.float32).reshape(-1)[:N_FEAT]
    wvec = np.tile(w8[None, :], (128, 1)).astype(np.float32)

    in_maps = []
    for c in range(N_CORES):
        in_maps.append({
            "data": np.ascontiguousarray(data_all[c]),
            "wvec": wvec,
        })
    return in_maps, label


def unshard(results, label):
    outs = [np.asarray(r["out"]).reshape(-1) for r in results]
    full = np.concatenate(outs)
    return full[label].astype(np.float32)


_CACHE = {}


def kernel(features, values, a0_weight, rows, num_nodes):
    assert int(num_nodes) == NUM_NODES
    in_maps, label = make_in_maps(features, values, a0_weight, rows)
    if "nc" not in _CACHE:
        _CACHE["nc"] = build_nc()
    nc = _CACHE["nc"]
    res = run_bass_kernel_spmd(nc, in_maps, core_ids=list(range(N_CORES)))
    return unshard(res.results, label)


# revision 6
# speedup vs baseline: 1.0444x; 1.0444x over previous
"""Trainium2 Bass kernel v5 for BERT4ETH adjacency build (FOLD design).

v3/v4 lineage (see kernel_v3.py docstring for the fold layout), plus:
  - bf16 features/values on the wire (halves the dominant DMA term);
    d is still accumulated in f32, so only input quantization error
    (~1e-3 relative) is introduced.
  - three-engine pipeline per chunk: DVE does the 5 weighted-feature
    MAC passes (scalar-AP ops are DVE-only in walrus), GPSIMD (Pool)
    does the value multiply and all the fold adds (plain tensor_tensor
    ops), ACT compacts the results into the accumulator.
  - triple-buffered input slots, NCHUNK=8 big chunks (per-chunk fixed
    costs — sem propagation, DMA issue, cross-engine stalls — amortize
    over 2x more work than v5).
  - features and value merged into ONE interleaved stream (6 bf16 per
    edge slot: f0..f4, v), so each chunk needs a single input DMA.
"""

import numpy as np
import ml_dtypes

import concourse.bass as bass
import concourse.mybir as mybir
from concourse.bass_utils import run_bass_kernel_spmd

F32 = mybir.dt.float32
BF16 = mybir.dt.bfloat16

N_CORES = 8
NUM_NODES = 3_000_000
N_FEAT = 5
NLANE = N_FEAT + 1            # interleaved lanes per edge slot: f0..f4, v
NCHUNK = 8
NBUF = 2
# leaf-block widths: powers of two AND 3*2^m — a width-3*2^m block folds
# with m power-of-2 levels plus two final stride-w adds, cutting padding
# ~16% versus pure powers of two
WIDTHS = [1, 2, 3, 4, 6, 8, 12, 16, 24, 32]
NCLS = len(WIDTHS)
MAXD = NCLS                   # class count (kept name for pack loops)
ORDER = list(range(NCLS))
S_D = [10, 23, 43, 60, 128, 84, 43, 3, 1, 1]  # per-(chunk,part) capacity
W_D = WIDTHS

_widths = [W_D[D] * S_D[D] for D in ORDER]
_offs = np.concatenate([[0], np.cumsum(_widths)])
OFF_D = {D: int(_offs[i]) for i, D in enumerate(ORDER)}
END_D = {D: int(_offs[i + 1]) for i, D in enumerate(ORDER)}
K = int(_offs[-1])            # leaf columns per chunk
SUM_S = int(sum(S_D))
ACC_BASE = {D: int(np.cumsum([0] + [S_D[d] for d in ORDER])[i])
            for i, D in enumerate(ORDER)}
COLS = NCHUNK * K
ACC = NCHUNK * SUM_S


def _emit_macs(eng, V, pcnt_ref, wait_ge, A, Bs, fg, w_sb):
    """A = sum_f w_f f_f in bf16.  All 5 tensor_scalar products go to
    independent buffers and issue back-to-back (engine stays fed); the 4
    adds use TARGETED wait counts (product f + previous add only), so the
    serial critical path is just the add chain."""
    base = pcnt_ref[0]
    V(eng.tensor_scalar(
        A, fg[:, 0:K], w_sb[:, 0:1], None, mybir.AluOpType.mult,
    ))
    for f in range(1, N_FEAT):
        V(eng.tensor_scalar(
            Bs[f - 1], fg[:, f * K : (f + 1) * K], w_sb[:, f : f + 1], None,
            mybir.AluOpType.mult,
        ))
    for f in range(1, N_FEAT):
        # add_f needs ts_b_f (count base+1+f) and add_{f-1} (count
        # base+N_FEAT+f-1, emitted after all N_FEAT products)
        if f == 1:
            wait_ge(base + 2)
        else:
            wait_ge(base + N_FEAT + f - 1)
        V(eng.tensor_tensor(
            out=A, in0=A, in1=Bs[f - 1], op=mybir.AluOpType.add,
        ))


def _fold_ops(w):
    """(step, in1_offset) pairs reducing a width-w node block in place;
    w is 2^m or 3*2^m.  After all ops the sum sits at offset 0 mod w."""
    ops = []
    m = 0
    while w % 2 == 0:
        m += 1
        w //= 2
        ops.append((1 << m, 1 << (m - 1)))
    if w == 3:
        full = 3 << m
        ops.append((full, 1 << m))
        ops.append((full, 2 << m))
    else:
        assert w == 1
    return ops


def _emit_folds(eng, V, Wt, X, classes, done_sem):
    """Strided fold adds for `classes`, emitted round-robin by fold level
    (one Wt per round); last op incs done_sem instead."""
    chains = {D: _fold_ops(W_D[D]) for D in classes if S_D[D] > 0}
    folds = []
    r = 0
    while True:
        level = [(D, chains[D][r]) for D in classes
                 if D in chains and r < len(chains[D])]
        if not level:
            break
        folds.append(level)
        r += 1
    flat = [(D, op) for level in folds for (D, op) in level]
    i = 0
    for level in folds:
        Wt()
        for D, (step, half) in level:
            off, end = OFF_D[D], END_D[D]
            inst = eng.tensor_tensor(
                out=X[:, off:end:step],
                in0=X[:, off:end:step],
                in1=X[:, off + half : end : step],
                op=mybir.AluOpType.add,
            )
            i += 1
            if i == len(flat):
                inst.then_inc(done_sem, 1)
            else:
                V(inst)


def build_nc(nchunk=NCHUNK, repeat=1):
    nc = bass.Bass()
    data = nc.dram_tensor("data", [128, COLS * NLANE], BF16, kind="ExternalInput")
    wvec = nc.dram_tensor("wvec", [128, 8], F32, kind="ExternalInput")
    out = nc.dram_tensor("out", [128, ACC], F32, kind="ExternalOutput")

    from contextlib import ExitStack
    ctx = ExitStack()
    with ctx:
        w_sb = ctx.enter_context(nc.sbuf_tensor("w_sb", [128, 8], F32))
        f_all = ctx.enter_context(
            nc.sbuf_tensor("f_sb", [128, NBUF * K * NLANE], BF16)
        )
        d_all = ctx.enter_context(nc.sbuf_tensor("d_sb", [128, NBUF * K], F32))
        a_all = ctx.enter_context(nc.sbuf_tensor("a_sb", [128, NBUF * K], BF16))
        b_all = ctx.enter_context(
            nc.sbuf_tensor("b_sb", [128, (N_FEAT - 1) * K], BF16)
        )
        accum = ctx.enter_context(nc.sbuf_tensor("accum", [128, ACC], F32))
        f_sb = [
            f_all[:, i * K * NLANE : (i + 1) * K * NLANE] for i in range(NBUF)
        ]
        d_sb = [d_all[:, i * K : (i + 1) * K] for i in range(NBUF)]
        a_sb = [a_all[:, i * K : (i + 1) * K] for i in range(NBUF)]
        b_sb = [b_all[:, i * K : (i + 1) * K] for i in range(N_FEAT - 1)]
        s_const = ctx.enter_context(nc.semaphore("s_const"))
        s_din = ctx.enter_context(nc.semaphore("s_din"))
        s_prep = ctx.enter_context(nc.semaphore("s_prep"))
        s_prepp = ctx.enter_context(nc.semaphore("s_prepp"))
        s_foldp = ctx.enter_context(nc.semaphore("s_foldp"))
        s_done = ctx.enter_context(nc.semaphore("s_done"))
        s_dout = ctx.enter_context(nc.semaphore("s_dout"))
        block = ctx.enter_context(nc.Block())

        @block.sync
        def _(sync):
            sync.dma_start(out=w_sb[:], in_=wvec[:]).then_inc(s_const, 16)
            for cc in range(nchunk * repeat):
                c = cc % nchunk
                s = cc % NBUF
                if cc >= NBUF:
                    sync.wait_ge(s_done, cc - NBUF + 1)
                sync.dma_start(
                    out=f_sb[s],
                    in_=data[:, c * K * NLANE : (c + 1) * K * NLANE],
                ).then_inc(s_din, 16)
            sync.wait_ge(s_done, nchunk * repeat)
            sync.dma_start(out=out[:], in_=accum[:]).then_inc(s_dout, 16)
            sync.wait_ge(s_dout, 16)

        @block.vector
        def _(vector):
            pcnt = 0

            def V(inst):
                nonlocal pcnt
                inst.then_inc(s_prep, 1)
                pcnt += 1

            def Wt():
                vector.wait_ge(s_prep, pcnt)

            vector.wait_ge(s_const, 16)
            for cc in range(nchunk * repeat):
                s = cc % NBUF
                vector.wait_ge(s_din, 16 * (cc + 1))
                _emit_macs(nc.vector, V, [pcnt],
                           lambda n: vector.wait_ge(s_prep, n),
                           a_sb[s], b_sb, f_sb[s], w_sb)
            assert pcnt == (2 * N_FEAT - 1) * nchunk * repeat

        @block.gpsimd
        def _(gpsimd):
            qcnt = 0

            def V(inst):
                nonlocal qcnt
                inst.then_inc(s_prepp, 1)
                qcnt += 1

            def Wt():
                gpsimd.wait_ge(s_prepp, qcnt)

            all_classes = [D for D in ORDER if S_D[D] > 0]
            for cc in range(nchunk * repeat):
                s = cc % NBUF
                # all 5 MAC passes of chunk cc retired on DVE
                gpsimd.wait_ge(s_prep, (2 * N_FEAT - 1) * (cc + 1))
                X = d_sb[s]
                V(nc.gpsimd.tensor_tensor(
                    out=X, in0=a_sb[s],
                    in1=f_sb[s][:, N_FEAT * K : NLANE * K],
                    op=mybir.AluOpType.mult
                ))
                _emit_folds(nc.gpsimd, V, Wt, X, all_classes, s_foldp)

        @block.scalar
        def _(scalar):
            last_D = ORDER[-1]
            for cc in range(nchunk * repeat):
                c = cc % nchunk
                s = cc % NBUF
                scalar.wait_ge(s_foldp, cc + 1)
                X = d_sb[s]
                aoff = c * SUM_S
                for D in ORDER:
                    if S_D[D] == 0:
                        continue
                    so = aoff + ACC_BASE[D]
                    inst = nc.scalar.copy(
                        accum[:, so : so + S_D[D]],
                        X[:, OFF_D[D] : END_D[D] : W_D[D]],
                    )
                    if D == last_D:
                        inst.then_inc(s_done, 1)

    return nc


# ---------------------------------------------------------------------------
# Host-side packing / unpacking
# ---------------------------------------------------------------------------

def _depth(c):
    return np.searchsorted(np.array(WIDTHS), np.maximum(c, 1))


def pack(rows):
    rows = np.asarray(rows, dtype=np.int64)
    counts = np.bincount(rows, minlength=NUM_NODES).astype(np.int64)
    if counts.max() > WIDTHS[-1]:
        raise RuntimeError(f"node count {counts.max()} exceeds max class width")

    order = np.argsort(-counts, kind="stable")
    node_core = np.empty(NUM_NODES, dtype=np.int64)
    node_core[order] = np.arange(NUM_NODES) % N_CORES

    depth = _depth(counts)

    node_chunk = np.empty(NUM_NODES, dtype=np.int64)
    node_part = np.empty(NUM_NODES, dtype=np.int64)
    node_idx = np.empty(NUM_NODES, dtype=np.int64)
    for core in range(N_CORES):
        sel_core = node_core == core
        for D in range(MAXD):
            nodes = order[sel_core[order] & (depth[order] == D)]
            n = len(nodes)
            if n > S_D[D] * 128 * NCHUNK:
                raise RuntimeError(
                    f"core {core} class {D}: {n} nodes > capacity "
                    f"{S_D[D] * 128 * NCHUNK}"
                )
            r = np.arange(n)
            node_part[nodes] = r % 128
            node_chunk[nodes] = (r // 128) % NCHUNK
            node_idx[nodes] = r // (128 * NCHUNK)

    off_arr = np.array([OFF_D[D] for D in range(MAXD)])
    accb_arr = np.array([ACC_BASE[D] for D in range(MAXD)])
    node_leafcol = (
        node_chunk * K + off_arr[depth] + node_idx * np.array(W_D)[depth]
    )
    node_acccol = node_chunk * SUM_S + accb_arr[depth] + node_idx

    eorder = np.argsort(rows, kind="stable")
    rs = rows[eorder]
    starts = np.zeros(NUM_NODES, dtype=np.int64)
    starts[1:] = np.cumsum(counts)[:-1]
    j = np.arange(len(rows), dtype=np.int64) - starts[rs]
    ecol = node_leafcol[rs] + j
    epart = node_part[rs]
    gpos = node_core[rs] * (128 * COLS) + epart * COLS + ecol

    label = node_core * (128 * ACC) + node_part * ACC + node_acccol
    return eorder, gpos, label


def make_in_maps(features, values, a0_weight, rows):
    features = np.asarray(features, dtype=np.float32)
    values = np.asarray(values, dtype=np.float32)
    eorder, gpos, label = pack(rows)

    ecore = gpos // (128 * COLS)
    epart = (gpos // COLS) % 128
    ecol = gpos % COLS
    echunk = ecol // K
    ej = ecol % K
    data_all = np.zeros((N_CORES, 128, NCHUNK, NLANE, K), dtype=np.float32)
    fsrc = features[eorder]
    for f in range(N_FEAT):
        data_all[ecore, epart, echunk, f, ej] = fsrc[:, f]
    data_all[ecore, epart, echunk, N_FEAT, ej] = values[eorder]
    data_all = data_all.astype(ml_dtypes.bfloat16).reshape(
        N_CORES, 128, COLS * NLANE
    )

    w8 = np.zeros(8, dtype=np.float32)
    w8[:N_FEAT] = np.asarray(a0_weight, dtype=np.float32).reshape(-1)[:N_FEAT]
    wvec = np.tile(w8[None, :], (128, 1)).astype(np.float32)

    in_maps = []
    for c in range(N_CORES):
        in_maps.append({
            "data": np.ascontiguousarray(data_all[c]),
            "wvec": wvec,
        })
    return in_maps, label


def unshard(results, label):
    outs = [np.asarray(r["out"]).reshape(-1) for r in results]
    full = np.concatenate(outs)
    return full[label].astype(np.float32)


_CACHE = {}


def kernel(features, values, a0_weight, rows, num_nodes):
    assert int(num_nodes) == NUM_NODES
    in_maps, label = make_in_maps(features, values, a0_weight, rows)
    if "nc" not in _CACHE:
        _CACHE["nc"] = build_nc()
    nc = _CACHE["nc"]
    res = run_bass_kernel_spmd(nc, in_maps, core_ids=list(range(N_CORES)))
    return unshard(res.results, label)


# revision 7
# speedup vs baseline: 1.2140x; 1.1624x over previous
"""Trainium2 Bass kernel v5 for BERT4ETH adjacency build (FOLD design).

v3/v4 lineage (see kernel_v3.py docstring for the fold layout), plus:
  - bf16 features/values on the wire (halves the dominant DMA term);
    d is still accumulated in f32, so only input quantization error
    (~1e-3 relative) is introduced.
  - three-engine pipeline per chunk: DVE does the 5 weighted-feature
    MAC passes (scalar-AP ops are DVE-only in walrus), GPSIMD (Pool)
    does the value multiply and all the fold adds (plain tensor_tensor
    ops), ACT compacts the results into the accumulator.
  - triple-buffered input slots, NCHUNK=8 big chunks (per-chunk fixed
    costs — sem propagation, DMA issue, cross-engine stalls — amortize
    over 2x more work than v5).
  - features and value merged into ONE interleaved stream (6 bf16 per
    edge slot: f0..f4, v), so each chunk needs a single input DMA.
"""

import numpy as np
import ml_dtypes

import concourse.bass as bass
import concourse.mybir as mybir
from concourse.bass_utils import run_bass_kernel_spmd

F32 = mybir.dt.float32
BF16 = mybir.dt.bfloat16

N_CORES = 8
NUM_NODES = 3_000_000
N_FEAT = 5
NLANE = N_FEAT + 1            # interleaved lanes per edge slot: f0..f4, v
NCHUNK = 8
NBUF = 3
# leaf-block widths: powers of two AND 3*2^m — a width-3*2^m block folds
# with m power-of-2 levels plus two final stride-w adds, cutting padding
# ~16% versus pure powers of two
WIDTHS = [1, 2, 3, 4, 6, 8, 12, 16, 24, 32]
NCLS = len(WIDTHS)
MAXD = NCLS                   # class count (kept name for pack loops)
ORDER = list(range(NCLS))
S_D = [10, 22, 41, 57, 123, 81, 41, 2, 1, 0]  # per-(chunk,part) capacity
W_D = WIDTHS

_widths = [W_D[D] * S_D[D] for D in ORDER]
_offs = np.concatenate([[0], np.cumsum(_widths)])
OFF_D = {D: int(_offs[i]) for i, D in enumerate(ORDER)}
END_D = {D: int(_offs[i + 1]) for i, D in enumerate(ORDER)}
K = int(_offs[-1])            # leaf columns per chunk
SUM_S = int(sum(S_D))
ACC_BASE = {D: int(np.cumsum([0] + [S_D[d] for d in ORDER])[i])
            for i, D in enumerate(ORDER)}
COLS = NCHUNK * K
ACC = NCHUNK * SUM_S


def _emit_macs(eng, V, pcnt_ref, wait_ge, A, Bs, fg, w_sb):
    """A = sum_f w_f f_f in bf16.  All 5 tensor_scalar products go to
    independent buffers and issue back-to-back (engine stays fed); the 4
    adds use TARGETED wait counts (product f + previous add only), so the
    serial critical path is just the add chain."""
    base = pcnt_ref[0]
    V(eng.tensor_scalar(
        A, fg[:, 0:K], w_sb[:, 0:1], None, mybir.AluOpType.mult,
    ))
    for f in range(1, N_FEAT):
        V(eng.tensor_scalar(
            Bs[f - 1], fg[:, f * K : (f + 1) * K], w_sb[:, f : f + 1], None,
            mybir.AluOpType.mult,
        ))
    for f in range(1, N_FEAT):
        # add_f needs ts_b_f (count base+1+f) and add_{f-1} (count
        # base+N_FEAT+f-1, emitted after all N_FEAT products)
        if f == 1:
            wait_ge(base + 2)
        else:
            wait_ge(base + N_FEAT + f - 1)
        V(eng.tensor_tensor(
            out=A, in0=A, in1=Bs[f - 1], op=mybir.AluOpType.add,
        ))


def _fold_ops(w):
    """(step, in1_offset) pairs reducing a width-w node block in place;
    w is 2^m or 3*2^m.  After all ops the sum sits at offset 0 mod w."""
    ops = []
    m = 0
    while w % 2 == 0:
        m += 1
        w //= 2
        ops.append((1 << m, 1 << (m - 1)))
    if w == 3:
        full = 3 << m
        ops.append((full, 1 << m))
        ops.append((full, 2 << m))
    else:
        assert w == 1
    return ops


def _emit_folds(eng, V, Wt, X, classes, done_sem):
    """Strided fold adds for `classes`, emitted round-robin by fold level
    (one Wt per round); last op incs done_sem instead."""
    chains = {D: _fold_ops(W_D[D]) for D in classes if S_D[D] > 0}
    folds = []
    r = 0
    while True:
        level = [(D, chains[D][r]) for D in classes
                 if D in chains and r < len(chains[D])]
        if not level:
            break
        folds.append(level)
        r += 1
    flat = [(D, op) for level in folds for (D, op) in level]
    i = 0
    for level in folds:
        Wt()
        for D, (step, half) in level:
            off, end = OFF_D[D], END_D[D]
            inst = eng.tensor_tensor(
                out=X[:, off:end:step],
                in0=X[:, off:end:step],
                in1=X[:, off + half : end : step],
                op=mybir.AluOpType.add,
            )
            i += 1
            if i == len(flat):
                inst.then_inc(done_sem, 1)
            else:
                V(inst)


def build_nc(nchunk=NCHUNK, repeat=1):
    nc = bass.Bass()
    data = nc.dram_tensor("data", [128, COLS * NLANE], BF16, kind="ExternalInput")
    wvec = nc.dram_tensor("wvec", [128, 8], F32, kind="ExternalInput")
    out = nc.dram_tensor("out", [128, ACC], F32, kind="ExternalOutput")

    from contextlib import ExitStack
    ctx = ExitStack()
    with ctx:
        w_sb = ctx.enter_context(nc.sbuf_tensor("w_sb", [128, 8], F32))
        f_all = ctx.enter_context(
            nc.sbuf_tensor("f_sb", [128, NBUF * K * NLANE], BF16)
        )
        d_all = ctx.enter_context(nc.sbuf_tensor("d_sb", [128, NBUF * K], F32))
        a_all = ctx.enter_context(nc.sbuf_tensor("a_sb", [128, NBUF * K], BF16))
        b_all = ctx.enter_context(
            nc.sbuf_tensor("b_sb", [128, (N_FEAT - 1) * K], BF16)
        )
        accum = ctx.enter_context(nc.sbuf_tensor("accum", [128, ACC], F32))
        f_sb = [
            f_all[:, i * K * NLANE : (i + 1) * K * NLANE] for i in range(NBUF)
        ]
        d_sb = [d_all[:, i * K : (i + 1) * K] for i in range(NBUF)]
        a_sb = [a_all[:, i * K : (i + 1) * K] for i in range(NBUF)]
        b_sb = [b_all[:, i * K : (i + 1) * K] for i in range(N_FEAT - 1)]
        s_const = ctx.enter_context(nc.semaphore("s_const"))
        s_din = ctx.enter_context(nc.semaphore("s_din"))
        s_prep = ctx.enter_context(nc.semaphore("s_prep"))
        s_prepp = ctx.enter_context(nc.semaphore("s_prepp"))
        s_foldp = ctx.enter_context(nc.semaphore("s_foldp"))
        s_done = ctx.enter_context(nc.semaphore("s_done"))
        s_dout = ctx.enter_context(nc.semaphore("s_dout"))
        block = ctx.enter_context(nc.Block())

        @block.sync
        def _(sync):
            sync.dma_start(out=w_sb[:], in_=wvec[:]).then_inc(s_const, 16)
            for cc in range(nchunk * repeat):
                c = cc % nchunk
                s = cc % NBUF
                if cc >= NBUF:
                    sync.wait_ge(s_done, cc - NBUF + 1)
                sync.dma_start(
                    out=f_sb[s],
                    in_=data[:, c * K * NLANE : (c + 1) * K * NLANE],
                ).then_inc(s_din, 16)
            sync.wait_ge(s_done, nchunk * repeat)
            sync.dma_start(out=out[:], in_=accum[:]).then_inc(s_dout, 16)
            sync.wait_ge(s_dout, 16)

        @block.vector
        def _(vector):
            pcnt = 0

            def V(inst):
                nonlocal pcnt
                inst.then_inc(s_prep, 1)
                pcnt += 1

            def Wt():
                vector.wait_ge(s_prep, pcnt)

            vector.wait_ge(s_const, 16)
            for cc in range(nchunk * repeat):
                s = cc % NBUF
                vector.wait_ge(s_din, 16 * (cc + 1))
                _emit_macs(nc.vector, V, [pcnt],
                           lambda n: vector.wait_ge(s_prep, n),
                           a_sb[s], b_sb, f_sb[s], w_sb)
            assert pcnt == (2 * N_FEAT - 1) * nchunk * repeat

        @block.gpsimd
        def _(gpsimd):
            qcnt = 0

            def V(inst):
                nonlocal qcnt
                inst.then_inc(s_prepp, 1)
                qcnt += 1

            def Wt():
                gpsimd.wait_ge(s_prepp, qcnt)

            all_classes = [D for D in ORDER if S_D[D] > 0]
            for cc in range(nchunk * repeat):
                s = cc % NBUF
                # all 5 MAC passes of chunk cc retired on DVE
                gpsimd.wait_ge(s_prep, (2 * N_FEAT - 1) * (cc + 1))
                X = d_sb[s]
                V(nc.gpsimd.tensor_tensor(
                    out=X, in0=a_sb[s],
                    in1=f_sb[s][:, N_FEAT * K : NLANE * K],
                    op=mybir.AluOpType.mult
                ))
                _emit_folds(nc.gpsimd, V, Wt, X, all_classes, s_foldp)

        @block.scalar
        def _(scalar):
            last_D = max(D for D in ORDER if S_D[D] > 0)
            for cc in range(nchunk * repeat):
                c = cc % nchunk
                s = cc % NBUF
                scalar.wait_ge(s_foldp, cc + 1)
                X = d_sb[s]
                aoff = c * SUM_S
                for D in ORDER:
                    if S_D[D] == 0:
                        continue
                    so = aoff + ACC_BASE[D]
                    inst = nc.scalar.copy(
                        accum[:, so : so + S_D[D]],
                        X[:, OFF_D[D] : END_D[D] : W_D[D]],
                    )
                    if D == last_D:
                        inst.then_inc(s_done, 1)

    return nc


# ---------------------------------------------------------------------------
# Host-side packing / unpacking
# ---------------------------------------------------------------------------

def _depth(c):
    return np.searchsorted(np.array(WIDTHS), np.maximum(c, 1))


def pack(rows):
    rows = np.asarray(rows, dtype=np.int64)
    counts = np.bincount(rows, minlength=NUM_NODES).astype(np.int64)
    if counts.max() > WIDTHS[-1]:
        raise RuntimeError(f"node count {counts.max()} exceeds max class width")

    order = np.argsort(-counts, kind="stable")
    node_core = np.empty(NUM_NODES, dtype=np.int64)
    node_core[order] = np.arange(NUM_NODES) % N_CORES

    depth = _depth(counts)

    node_chunk = np.empty(NUM_NODES, dtype=np.int64)
    node_part = np.empty(NUM_NODES, dtype=np.int64)
    node_idx = np.empty(NUM_NODES, dtype=np.int64)
    for core in range(N_CORES):
        sel_core = node_core == core
        for D in range(MAXD):
            nodes = order[sel_core[order] & (depth[order] == D)]
            n = len(nodes)
            if n > S_D[D] * 128 * NCHUNK:
                raise RuntimeError(
                    f"core {core} class {D}: {n} nodes > capacity "
                    f"{S_D[D] * 128 * NCHUNK}"
                )
            r = np.arange(n)
            node_part[nodes] = r % 128
            node_chunk[nodes] = (r // 128) % NCHUNK
            node_idx[nodes] = r // (128 * NCHUNK)

    off_arr = np.array([OFF_D[D] for D in range(MAXD)])
    accb_arr = np.array([ACC_BASE[D] for D in range(MAXD)])
    node_leafcol = (
        node_chunk * K + off_arr[depth] + node_idx * np.array(W_D)[depth]
    )
    node_acccol = node_chunk * SUM_S + accb_arr[depth] + node_idx

    eorder = np.argsort(rows, kind="stable")
    rs = rows[eorder]
    starts = np.zeros(NUM_NODES, dtype=np.int64)
    starts[1:] = np.cumsum(counts)[:-1]
    j = np.arange(len(rows), dtype=np.int64) - starts[rs]
    ecol = node_leafcol[rs] + j
    epart = node_part[rs]
    gpos = node_core[rs] * (128 * COLS) + epart * COLS + ecol

    label = node_core * (128 * ACC) + node_part * ACC + node_acccol
    return eorder, gpos, label


def make_in_maps(features, values, a0_weight, rows):
    features = np.asarray(features, dtype=np.float32)
    values = np.asarray(values, dtype=np.float32)
    eorder, gpos, label = pack(rows)

    ecore = gpos // (128 * COLS)
    epart = (gpos // COLS) % 128
    ecol = gpos % COLS
    echunk = ecol // K
    ej = ecol % K
    data_all = np.zeros((N_CORES, 128, NCHUNK, NLANE, K), dtype=np.float32)
    fsrc = features[eorder]
    for f in range(N_FEAT):
        data_all[ecore, epart, echunk, f, ej] = fsrc[:, f]
    data_all[ecore, epart, echunk, N_FEAT, ej] = values[eorder]
    data_all = data_all.astype(ml_dtypes.bfloat16).reshape(
        N_CORES, 128, COLS * NLANE
    )

    w8 = np.zeros(8, dtype=np.float32)
    w8[:N_FEAT] = np.asarray(a0_weight, dtype=np.float32).reshape(-1)[:N_FEAT]
    wvec = np.tile(w8[None, :], (128, 1)).astype(np.float32)

    in_maps = []
    for c in range(N_CORES):
        in_maps.append({
            "data": np.ascontiguousarray(data_all[c]),
            "wvec": wvec,
        })
    return in_maps, label


def unshard(results, label):
    outs = [np.asarray(r["out"]).reshape(-1) for r in results]
    full = np.concatenate(outs)
    return full[label].astype(np.float32)


_CACHE = {}


def kernel(features, values, a0_weight, rows, num_nodes):
    assert int(num_nodes) == NUM_NODES
    in_maps, label = make_in_maps(features, values, a0_weight, rows)
    if "nc" not in _CACHE:
        _CACHE["nc"] = build_nc()
    nc = _CACHE["nc"]
    res = run_bass_kernel_spmd(nc, in_maps, core_ids=list(range(N_CORES)))
    return unshard(res.results, label)
